# revision 22
# baseline (speedup 1.0000x reference)
"""Trainium2 Bass kernel for the GatedCRF 3D semseg loss.

Reformulation (validated vs reference to ~6e-7 rel in fp64):
With C=2 softmax channels, y0+y1=1. Let a = 1-2*y0, then per voxel-pair
  y0A*y1B + y1A*y0B = (1 - aA*aB)/2
so with E(l,delta) = exp(-0.5*((I[l+d]-I[l])/SIMG)^2 - 0.5*msq(delta)):
  loss*denom = sum_{d in HALF} [ sum_l E  -  sum_l E*aA*aB ] + G_total
where HALF is the 73 lexicographically-positive offsets of the 7x7x3
window and G_total is the out-of-bounds kernel mass
sum_l noob(l)*exp(-0.5*msq_c(l) - 0.5*(I_l/SIMG)^2).

Validity masking is data-driven: out-of-volume halo voxels carry J = BIG
so any one-sided-OOB pair gets E = exp(-huge) = 0, and both-OOB pairs
have aA = aB = 1 (u=0 pad) so E - E*aA*aB cancels exactly.

sum_l E rides free on the Exp's accum_out. The product side factors as
sum_l aA * Q(l) with Q = sum_d E_d * aB_d accumulated across slots by
in-place DVE adds (first pair's v-mult writes Q directly), so each
offset needs only {sub, Square, Exp, v-mult, Q+=v} and the product
reduction is ONE final STT-with-accum.

Engine notes (measured on HW):
 - GPSIMD shares its SBUF port with the DVE; running it concurrently
   slows DVE 3.5x -> everything stays on DVE+ACT.
 - ACT is 1x-rate ((FD+352)/1.2GHz) regardless of dtype.
 - DVE fp16 TT with step-1 4B-aligned operands runs 2x
   ((FD/2+151)/0.96GHz); STT runs 1x. In-place accumulate TTs and
   3-free-dim register APs with stride-0 broadcast all run at full 2x.
 - DMA: ~300GB/s per queue with ~13KB/partition rows, ~115GB/s with
   half that, so J and a ship as ONE combined dual-copy array each
   ([X | X<<1elem], fp16, pad included) split across the two HW DGE
   queues (SP + ACT) by partition halves.

J/a dual copies make every dynamic window shift resolve to a
4B-aligned base (joff even -> copy0, odd -> copy1 at joff-1), keeping
the 2x mode engaged. dd=+-1 offset pairs share one bias (dd^2) and are
fused per-slot via a stride-2 AP dim with a stride-0 broadcast A-side.
Layout: partition p = 16*h_blk + w_blk is a (4h x 8w) block with
halos; only dh >= 0 is ever read (top h-halo trimmed: 7 stored rows).

Per-core slots (SPMD; offsets/biases are per-core data):
3 dd=+-1 pair slots (6 offsets) + 3 dd=0 single slots + 1/8 of the
lone (0,0,1) offset (spatially split mini-slot) + 1/8 of the G-pass.
"""

import numpy as np

# problem constants (hardcoded per contract)
H, W, D = 64, 64, 32
SXY, SIMG = 5.0, 0.1
RH, RW, RD = 3, 3, 1
NCORES = 8
NPAIR, NSING = 3, 3
BH, BW = 4, 8                    # central block per partition
NHB, NWB = H // BH, W // BW      # 16 x 8 blocks = 128 partitions
SH = BH + RH                     # 7 stored h rows (top halo trimmed)
SW = BW + 2 * RW                 # 14
SD = D + 2 * RD                  # 34
FREE = SH * SW * SD              # 3332 stored elems per partition
ROW = BW * SD                    # 272: fused (w,d) run per h row
PAD = 8
JP = 2 * FREE + PAD              # dual-copy row length (6672, even)
CENT = RW * SD + RD              # 103 (odd) central base in copy-0
CENT1 = FREE + CENT - 1          # 3434 (even) central base in copy-1
MROW = BH * ROW // NCORES        # 136: mini/G slice length per core
NQ = BH * ROW                    # 1088
NCOLS = 12                       # E sums (7) | spare | Sfin | Smini | G
NMETA = 136 + 7 + 9              # t3 slice | biases | int offsets
SQ = float(np.sqrt(0.5) / SIMG)  # sqrt(50)
S2 = float(0.5 / SIMG ** 2)      # 50
BIG = 28.0                       # halo marker: max |d|=BIG+6 -> q<=57800
NEG = -1.0e4
DENOM = float(H * W * D)
OFFB = RH * SW * SD + 2 * RW * SD + 2 * RD   # 1634: max slot base
HALF = 64                        # partition split point for dual-queue DMA


def _pair_single_slots():
    """pairs/singles = (dh,dw) lists; pairs fuse dd=+-1, singles dd=0."""
    pairs, singles = [], []
    for dh in range(0, RH + 1):
        for dw in range(-RW, RW + 1):
            if (dh > 0) or (dh == 0 and dw > 0):
                pairs.append((dh, dw))
                singles.append((dh, dw))
    assert len(pairs) == 24 and len(singles) == 24
    return pairs, singles


def _pack(v, pad_val):
    """(H, W, D) -> [128, FREE]: per-partition block + trimmed halos."""
    vp = np.pad(v.astype(np.float32), ((RH, RH), (RW, RW), (RD, RD)),
                constant_values=pad_val)
    out = np.empty((128, SH, SW, SD), np.float32)
    for hb in range(NHB):
        for wb in range(NWB):
            out[hb * NWB + wb] = vp[hb * BH + RH:hb * BH + RH + SH,
                                    wb * BW:wb * BW + SW, :]
    return out.reshape(128, FREE)


def _dual_f16(flat, tail):
    """[128, FREE] -> fp16 [128, JP] = [X | X<<1elem | zero pad]."""
    x = flat.astype(np.float16)
    out = np.zeros((128, JP), np.float16)
    out[:, :FREE] = x
    out[:, FREE:2 * FREE - 1] = x[:, 1:]
    out[:, 2 * FREE - 1] = tail
    return out


def _build_nc():
    import concourse.bass as bass
    import concourse.bacc as bacc
    import concourse.mybir as mybir
    from concourse.tile import TileContext

    f32, f16, i32 = mybir.dt.float32, mybir.dt.float16, mybir.dt.int32
    AF = mybir.ActivationFunctionType
    OP = mybir.AluOpType
    ET = mybir.EngineType

    nc = bacc.Bacc("TRN2", target_bir_lowering=False, debug=False)
    jd = nc.dram_tensor("jd", [128, JP], f16, kind="ExternalInput")
    ad = nc.dram_tensor("ad", [128, JP], f16, kind="ExternalInput")
    meta = nc.dram_tensor("meta", [128, NMETA], f32, kind="ExternalInput")
    out = nc.dram_tensor("out", [128, NCOLS], f32, kind="ExternalOutput")

    # patterns: [partition][(pair)][h][flat (w,d) row]
    P1 = [[JP, 128], [SW * SD, BH], [1, ROW]]                 # single B-view
    P2 = [[JP, 128], [2, 2], [SW * SD, BH], [1, ROW]]         # dd=+-1 pair
    P2B = [[JP, 128], [0, 2], [SW * SD, BH], [1, ROW]]        # bcast A-side
    PM = [[JP, 128], [1, MROW]]                               # mini slice

    with TileContext(nc) as tc:
        with tc.tile_pool(name="pers", bufs=1) as pers, \
             tc.tile_pool(name="dp2", bufs=2) as dp2, \
             tc.tile_pool(name="qp2", bufs=2) as qp2, \
             tc.tile_pool(name="ep2", bufs=2) as ep2, \
             tc.tile_pool(name="vp2", bufs=2) as vp2, \
             tc.tile_pool(name="dp1", bufs=2) as dp1, \
             tc.tile_pool(name="qp1", bufs=2) as qp1, \
             tc.tile_pool(name="ep1", bufs=2) as ep1, \
             tc.tile_pool(name="vp1", bufs=2) as vp1, \
             tc.tile_pool(name="gp", bufs=1) as gp:
            J = pers.tile([128, JP], f16, tag="J")
            A = pers.tile([128, JP], f16, tag="A")
            metatile = pers.tile([128, NMETA], f32, tag="meta")
            acc = pers.tile([128, NCOLS], f32, tag="acc")
            Q2 = pers.tile([128, 2 * NQ], f16, tag="Q2")
            fin = pers.tile([128, NQ], f16, tag="fin")

            nc.vector.memset(acc[:], 0.0)
            # dual-queue DMA: partition halves on SP and ACT HW DGE rings
            nc.sync.dma_start(metatile[:], meta[:])
            nc.sync.dma_start(J[0:HALF, :], jd[0:HALF, :])
            nc.scalar.dma_start(J[HALF:128, :], jd[HALF:128, :])
            nc.sync.dma_start(A[0:HALF, :], ad[0:HALF, :])
            nc.scalar.dma_start(A[HALF:128, :], ad[HALF:128, :])

            t3v = metatile[:, 0:MROW]
            BIAS0 = MROW
            biasv = metatile[:, BIAS0:BIAS0 + 7]   # pair0..2, sing0..2, mini
            INT0 = BIAS0 + 7
            # ints: pair joffs 0..2 | single joffs 3..5 | mini jB 6 |
            #       mini jA 7 | gjoff 8
            _, sv = nc.values_load_multi_w_load_instructions(
                metatile[0:1, INT0 + 3:INT0 + 6].bitcast(i32),
                engines=(ET.DVE,), min_val=FREE, max_val=FREE + OFFB,
                skip_runtime_bounds_check=True)
            _, pv = nc.values_load_multi_w_load_instructions(
                metatile[0:1, INT0:INT0 + 3].bitcast(i32),
                engines=(ET.DVE,), min_val=0, max_val=CENT + 1 + OFFB,
                skip_runtime_bounds_check=True)
            _, mv = nc.values_load_multi_w_load_instructions(
                metatile[0:1, INT0 + 6:INT0 + 8].bitcast(i32),
                engines=(ET.DVE,), min_val=0, max_val=FREE + CENT + OFFB,
                skip_runtime_bounds_check=True)
            gval = nc.values_load(
                metatile[0:1, INT0 + 8:INT0 + 9].bitcast(i32),
                engines=(ET.Activation,), min_val=FREE,
                max_val=FREE + CENT + OFFB,
                skip_runtime_bounds_check=True)

            J_A1 = bass.AP(J.tensor, CENT1, P1)
            J_A2 = bass.AP(J.tensor, CENT1, P2B)

            def pair_slot(j, first):
                dt = dp2.tile([128, 2, BH, ROW], f16, tag="d2")
                nc.vector.tensor_tensor(
                    dt[:], bass.AP(J.tensor, pv[j], P2), J_A2, OP.subtract)
                qt = qp2.tile([128, 2, BH, ROW], f16, tag="q2")
                nc.scalar.activation(qt[:], dt[:], AF.Square, scale=SQ)
                et = ep2.tile([128, 2, BH, ROW], f16, tag="e2")
                nc.scalar.activation(et[:], qt[:], AF.Exp, scale=-1.0,
                                     bias=biasv[:, j:j + 1],
                                     accum_out=acc[:, j:j + 1])
                if first:
                    nc.vector.tensor_tensor(
                        Q2[:].rearrange("p (a b c) -> p a b c", a=2, b=BH),
                        et[:], bass.AP(A.tensor, pv[j], P2), OP.mult)
                else:
                    vt = vp2.tile([128, 2, BH, ROW], f16, tag="v2")
                    nc.vector.tensor_tensor(
                        vt[:], et[:], bass.AP(A.tensor, pv[j], P2), OP.mult)
                    nc.vector.tensor_tensor(
                        Q2[:], Q2[:],
                        vt[:].rearrange("p a b c -> p (a b c)"), OP.add)

            def single_slot(j):
                dt = dp1.tile([128, BH, ROW], f16, tag="d1")
                nc.vector.tensor_tensor(
                    dt[:], bass.AP(J.tensor, sv[j], P1), J_A1, OP.subtract)
                qt = qp1.tile([128, BH, ROW], f16, tag="q1")
                if j == 0:
                    nc.vector.tensor_tensor(qt[:], dt[:], dt[:], OP.mult)
                    escale = -S2
                else:
                    nc.scalar.activation(qt[:], dt[:], AF.Square, scale=SQ)
                    escale = -1.0
                et = ep1.tile([128, BH, ROW], f16, tag="e1")
                nc.scalar.activation(et[:], qt[:], AF.Exp, scale=escale,
                                     bias=biasv[:, NPAIR + j:NPAIR + j + 1],
                                     accum_out=acc[:, NPAIR + j:NPAIR + j + 1])
                vt = vp1.tile([128, BH, ROW], f16, tag="v1")
                nc.vector.tensor_tensor(
                    vt[:], et[:], bass.AP(A.tensor, sv[j], P1), OP.mult)
                nc.vector.tensor_tensor(
                    Q2[:, 0:NQ], Q2[:, 0:NQ],
                    vt[:].rearrange("p a b -> p (a b)"), OP.add)

            # pairs first (Q2 initialized by pair0's v-mult); the lane
            # collapse overlaps the last single; end on short chains.
            pair_slot(0, True)
            single_slot(0)
            pair_slot(1, False)
            single_slot(1)
            pair_slot(2, False)

            # ---- mini slot: 1/8 of the lone (0,0,1) offset ----
            md = gp.tile([128, MROW], f16, tag="md")
            nc.vector.tensor_tensor(
                md[:], bass.AP(J.tensor, mv[0], PM),
                bass.AP(J.tensor, mv[1], PM), OP.subtract)
            mq = gp.tile([128, MROW], f16, tag="mq")
            nc.vector.tensor_tensor(mq[:], md[:], md[:], OP.mult)
            me = gp.tile([128, MROW], f16, tag="me")
            nc.scalar.activation(me[:], mq[:], AF.Exp, scale=-S2,
                                 bias=biasv[:, 6:7],
                                 accum_out=acc[:, 6:7])
            mvt = gp.tile([128, MROW], f16, tag="mv")
            nc.vector.tensor_tensor(
                mvt[:], me[:], bass.AP(A.tensor, mv[0], PM), OP.mult)
            ms = gp.tile([128, MROW], f16, tag="ms")
            nc.vector.scalar_tensor_tensor(
                ms[:], mvt[:], 1.0, bass.AP(A.tensor, mv[1], PM),
                OP.mult, OP.mult, accum_out=acc[:, 9:10])

            # ---- G-pass: 1/8 of central voxels per core ----
            qg = gp.tile([128, MROW], f16, tag="qg")
            nc.scalar.activation(
                qg[:], bass.AP(J.tensor, gval, PM), AF.Square, scale=SQ)
            ag = gp.tile([128, MROW], f32, tag="ag")
            nc.vector.scalar_tensor_tensor(
                ag[:], qg[:], -1.0, t3v[:, 0:MROW], OP.mult, OP.add)
            eg = gp.tile([128, MROW], f16, tag="eg")
            nc.scalar.activation(eg[:], ag[:], AF.Exp,
                                 accum_out=acc[:, 10:11])

            # lane collapse overlaps the last single slot
            nc.vector.tensor_tensor(Q2[:, 0:NQ], Q2[:, 0:NQ],
                                    Q2[:, NQ:2 * NQ], OP.add)
            single_slot(2)

            # ---- final: col8 = sum aA * (Q2 lane0 + lane1) ----
            a_A1 = bass.AP(A.tensor, CENT1, P1)
            nc.vector.scalar_tensor_tensor(
                fin[:].rearrange("p (a b) -> p a b", a=BH, b=ROW),
                Q2[:, 0:NQ].rearrange("p (a b) -> p a b", a=BH, b=ROW),
                1.0, a_A1, OP.mult, OP.mult, accum_out=acc[:, 8:9])

            nc.sync.dma_start(out[:], acc[:])
    nc.compile()
    return nc


def _host_tables(sample, spacing):
    """Per-core meta arrays."""
    sp = np.asarray(spacing, dtype=np.float64)[:, 0]
    pairs, singles = _pair_single_slots()

    # t3 = ln(noob) - 0.5*msq_center (NEG where noob == 0), central packing
    h = np.arange(H)[:, None, None]
    w = np.arange(W)[None, :, None]
    d = np.arange(D)[None, None, :]
    msq_c = ((sp[0] * h) ** 2 + (sp[1] * w) ** 2 + (sp[2] * d) ** 2) / SXY ** 2
    cnt = ((np.minimum(h, RH) + np.minimum(H - 1 - h, RH) + 1)
           * (np.minimum(w, RW) + np.minimum(W - 1 - w, RW) + 1)
           * (np.minimum(d, RD) + np.minimum(D - 1 - d, RD) + 1))
    noob = (2 * RH + 1) * (2 * RW + 1) * (2 * RD + 1) - cnt
    t3full = np.where(noob > 0, np.log(np.maximum(noob, 1)) - 0.5 * msq_c, NEG)
    t3b = np.empty((128, BH, BW, D), np.float32)
    for hb in range(NHB):
        for wb in range(NWB):
            t3b[hb * NWB + wb] = t3full[hb * BH:(hb + 1) * BH,
                                        wb * BW:(wb + 1) * BW, :]
    t3flat = t3b.reshape(128, BH * BW * D)

    def bias_of(dh, dw, dd):
        msq = ((sp[0] * dh) ** 2 + (sp[1] * dw) ** 2
               + (sp[2] * dd) ** 2) / SXY ** 2
        return -0.5 * msq

    metas = []
    for c in range(NCORES):
        m = np.zeros((128, NMETA), np.float32)
        # G/mini slice: h-row c//2, col half c%2 of the 272-wide flat row
        sl = (c // 2) * SW * SD + (c % 2) * MROW
        # map on-chip slice positions (d incl halo) to central t3 values
        t3s = np.full((128, MROW), NEG, np.float32)
        r0 = c // 2
        base = (c % 2) * MROW
        for i in range(MROW):
            wcol, dcol = divmod(base + i, SD)
            if 1 <= dcol <= D:
                t3s[:, i] = t3flat[:, (r0 * BW + wcol) * D + (dcol - 1)]
        m[:, 0:MROW] = t3s

        B0 = MROW
        joff_p = np.zeros(3, np.int32)
        joff_s = np.zeros(3, np.int32)
        for j in range(NPAIR):
            dh, dw = pairs[3 * c + j]
            j0 = dh * SW * SD + (RW + dw) * SD + (RD - 1)   # dd=-1 lane
            assert j0 % 2 == 0
            joff_p[j] = j0                                  # copy-0 coords
            m[:, B0 + j] = bias_of(dh, dw, 1)
        for j in range(NSING):
            dh, dw = singles[3 * c + j]
            j0 = dh * SW * SD + (RW + dw) * SD + RD         # dd=0
            assert j0 % 2 == 1
            joff_s[j] = FREE + j0 - 1                       # copy-1 coords
            m[:, B0 + NPAIR + j] = bias_of(dh, dw, 0)
        m[:, B0 + 6] = bias_of(0, 0, 1)                     # mini
        I0 = B0 + 7
        m[0, I0:I0 + 3] = joff_p.view(np.float32)
        m[0, I0 + 3:I0 + 6] = joff_s.view(np.float32)
        # mini: B = central+1 (dd=+1) in copy-0, A = central in copy-1
        mb = CENT + 1 + sl
        ma = CENT1 + sl
        assert mb % 2 == 0 and ma % 2 == 0
        m[0, I0 + 6] = np.int32(mb).view(np.float32)
        m[0, I0 + 7] = np.int32(ma).view(np.float32)
        m[0, I0 + 8] = np.int32(CENT1 + sl).view(np.float32)  # gjoff
        metas.append(m)
    return metas


def _host_inputs(y_hat_softmax, sample, spacing):
    y0 = np.asarray(y_hat_softmax, dtype=np.float32)[0, 0]
    I = np.asarray(sample, dtype=np.float32)[0, 0]
    jd = _dual_f16(_pack(I, BIG), 0.0)
    ad = _dual_f16(_pack(1.0 - 2.0 * y0, 1.0), 1.0)
    metas = _host_tables(sample, spacing)
    return [{"jd": jd, "ad": ad, "meta": metas[c]} for c in range(NCORES)]


def kernel(y_hat_softmax, sample, spacing):
    from concourse.bass_utils import run_bass_kernel_spmd

    in_maps = _host_inputs(y_hat_softmax, sample, spacing)
    nc = _build_nc()
    res = run_bass_kernel_spmd(nc, in_maps, core_ids=list(range(NCORES)))
    total = 0.0
    for r in res.results:
        o = r["out"].astype(np.float64)
        # cols 0..6 = sum E per slot (pairs, singles, mini); col8 = final
        # sum aA*Q2; col9 = mini product sum; col10 = G
        total += (o[:, 0:7].sum() - o[:, 8].sum() - o[:, 9].sum()
                  + o[:, 10].sum())
    return np.array(total / DENOM, dtype=np.float32)


if __name__ == "__main__":
    rng = np.random.default_rng(0)
    logits = rng.standard_normal((1, 2, H, W, D)).astype(np.float32)
    e = np.exp(logits - logits.max(axis=1, keepdims=True))
    yh = (e / e.sum(axis=1, keepdims=True)).astype(np.float32)
    smp = rng.standard_normal((1, 1, H, W, D)).astype(np.float32)
    spc = rng.uniform(0.5, 2.0, (3, 1)).astype(np.float32)
    print(kernel(yh, smp, spc))


# revision 25
# speedup vs baseline: 1.2422x; 1.2422x over previous
"""Trainium2 Bass kernel for the GatedCRF 3D semseg loss.

Reformulation (validated vs reference to ~6e-7 rel in fp64):
With C=2 softmax channels, y0+y1=1. Let a = 1-2*y0, then per voxel-pair
  y0A*y1B + y1A*y0B = (1 - aA*aB)/2
so with E(l,delta) = exp(-0.5*((I[l+d]-I[l])/SIMG)^2 - 0.5*msq(delta)):
  loss*denom = sum_{d in HALF} [ sum_l E  -  sum_l E*aA*aB ] + G_total
where HALF is the 73 lexicographically-positive offsets of the 7x7x3
window and G_total is the out-of-bounds kernel mass
sum_l noob(l)*exp(-0.5*msq_c(l) - 0.5*(I_l/SIMG)^2).

Validity masking is data-driven: out-of-volume halo voxels carry J = BIG
so any one-sided-OOB pair gets E = exp(-huge) = 0, and both-OOB pairs
have aA = aB = 1 (u=0 pad) so E - E*aA*aB cancels exactly.

sum_l E rides free on the Exp's accum_out. The product side factors as
sum_l aA * Q(l) with Q = sum_d E_d * aB_d accumulated across slots by
in-place DVE adds (first pair's v-mult writes Q directly), so each
offset needs only {sub, Square, Exp, v-mult, Q+=v} and the product
reduction is ONE final STT-with-accum.

Engine notes (measured on HW):
 - GPSIMD shares its SBUF port with the DVE; running it concurrently
   slows DVE 3.5x -> everything stays on DVE+ACT.
 - ACT is 1x-rate ((FD+352)/1.2GHz) regardless of dtype.
 - DVE fp16 TT with step-1 4B-aligned operands runs 2x
   ((FD/2+151)/0.96GHz); STT runs 1x. In-place accumulate TTs and
   3-free-dim register APs with stride-0 broadcast all run at full 2x.
 - DMA: ~300GB/s per queue with ~13KB/partition rows, ~115GB/s with
   half that, so J and a ship as ONE combined dual-copy array each
   ([X | X<<1elem], fp16, pad included) split across the two HW DGE
   queues (SP + ACT) by partition halves.

J/a dual copies make every dynamic window shift resolve to a
4B-aligned base (joff even -> copy0, odd -> copy1 at joff-1), keeping
the 2x mode engaged. dd=+-1 offset pairs share one bias (dd^2) and are
fused per-slot via a stride-2 AP dim with a stride-0 broadcast A-side.
Layout: partition p = 16*h_blk + w_blk is a (4h x 8w) block with
halos; only dh >= 0 is ever read (top h-halo trimmed: 7 stored rows).

Per-core slots (SPMD; offsets/biases are per-core data):
3 dd=+-1 pair slots (6 offsets) + 3 dd=0 single slots + 1/8 of the
lone (0,0,1) offset (spatially split mini-slot) + 1/8 of the G-pass.
"""

import numpy as np

# problem constants (hardcoded per contract)
H, W, D = 64, 64, 32
SXY, SIMG = 5.0, 0.1
RH, RW, RD = 3, 3, 1
NCORES = 8
NPAIR, NSING = 3, 3
BH, BW = 4, 8                    # central block per partition
NHB, NWB = H // BH, W // BW      # 16 x 8 blocks = 128 partitions
SH = BH + RH                     # 7 stored h rows (top halo trimmed)
SW = BW + 2 * RW                 # 14
SD = D + 2 * RD                  # 34
FREE = SH * SW * SD              # 3332 stored elems per partition
ROW = BW * SD                    # 272: fused (w,d) run per h row
PAD = 8
JP = 2 * FREE + PAD              # dual-copy row length (6672, even)
CENT = RW * SD + RD              # 103 (odd) central base in copy-0
CENT1 = FREE + CENT - 1          # 3434 (even) central base in copy-1
MROW = BH * ROW // NCORES        # 136: mini/G slice length per core
NQ = BH * ROW                    # 1088
NCOLS = 12                       # E sums(7) | spare | Sfin | Smini | G | S2
NMETA = 136 + 7 + 9              # t3 slice | biases | int offsets
SQ = float(np.sqrt(0.5) / SIMG)  # sqrt(50)
S2 = float(0.5 / SIMG ** 2)      # 50
BIG = 28.0                       # halo marker: max |d|=BIG+6 -> q<=57800
NEG = -1.0e4
DENOM = float(H * W * D)
OFFB = RH * SW * SD + 2 * RW * SD + 2 * RD   # 1634: max slot base
HALF = 64                        # partition split point for dual-queue DMA


def _pair_single_slots():
    """pairs/singles = (dh,dw) lists; pairs fuse dd=+-1, singles dd=0."""
    pairs, singles = [], []
    for dh in range(0, RH + 1):
        for dw in range(-RW, RW + 1):
            if (dh > 0) or (dh == 0 and dw > 0):
                pairs.append((dh, dw))
                singles.append((dh, dw))
    assert len(pairs) == 24 and len(singles) == 24
    return pairs, singles


def _pack(v, pad_val):
    """(H, W, D) -> [128, FREE]: per-partition block + trimmed halos."""
    vp = np.pad(v.astype(np.float32), ((RH, RH), (RW, RW), (RD, RD)),
                constant_values=pad_val)
    out = np.empty((128, SH, SW, SD), np.float32)
    for hb in range(NHB):
        for wb in range(NWB):
            out[hb * NWB + wb] = vp[hb * BH + RH:hb * BH + RH + SH,
                                    wb * BW:wb * BW + SW, :]
    return out.reshape(128, FREE)


def _dual_f16(flat, tail):
    """[128, FREE] -> fp16 [128, JP] = [X | X<<1elem | zero pad]."""
    x = flat.astype(np.float16)
    out = np.zeros((128, JP), np.float16)
    out[:, :FREE] = x
    out[:, FREE:2 * FREE - 1] = x[:, 1:]
    out[:, 2 * FREE - 1] = tail
    return out


def _build_nc():
    import concourse.bass as bass
    import concourse.bacc as bacc
    import concourse.mybir as mybir
    from concourse.tile import TileContext

    f32, f16, i32 = mybir.dt.float32, mybir.dt.float16, mybir.dt.int32
    AF = mybir.ActivationFunctionType
    OP = mybir.AluOpType
    ET = mybir.EngineType

    nc = bacc.Bacc("TRN2", target_bir_lowering=False, debug=False)
    jd = nc.dram_tensor("jd", [128, JP], f16, kind="ExternalInput")
    ad = nc.dram_tensor("ad", [128, JP], f16, kind="ExternalInput")
    meta = nc.dram_tensor("meta", [128, NMETA], f32, kind="ExternalInput")
    out = nc.dram_tensor("out", [128, NCOLS], f32, kind="ExternalOutput")

    # patterns: [partition][(pair)][h][flat (w,d) row]
    P1 = [[JP, 128], [SW * SD, BH], [1, ROW]]                 # single B-view
    P2 = [[JP, 128], [2, 2], [SW * SD, BH], [1, ROW]]         # dd=+-1 pair
    P2B = [[JP, 128], [0, 2], [SW * SD, BH], [1, ROW]]        # bcast A-side
    PM = [[JP, 128], [1, MROW]]                               # mini slice

    with TileContext(nc) as tc:
        with tc.tile_pool(name="pers", bufs=1) as pers, \
             tc.tile_pool(name="dp2", bufs=2) as dp2, \
             tc.tile_pool(name="qp2", bufs=2) as qp2, \
             tc.tile_pool(name="ep2", bufs=2) as ep2, \
             tc.tile_pool(name="vp2", bufs=2) as vp2, \
             tc.tile_pool(name="dp1", bufs=2) as dp1, \
             tc.tile_pool(name="qp1", bufs=2) as qp1, \
             tc.tile_pool(name="ep1", bufs=2) as ep1, \
             tc.tile_pool(name="vp1", bufs=2) as vp1, \
             tc.tile_pool(name="gp", bufs=1) as gp:
            J = pers.tile([128, JP], f16, tag="J")
            A = pers.tile([128, JP], f16, tag="A")
            metatile = pers.tile([128, NMETA], f32, tag="meta")
            acc = pers.tile([128, NCOLS], f32, tag="acc")
            Q1 = pers.tile([128, NQ], f16, tag="Q1")
            Q2 = pers.tile([128, 2 * NQ], f16, tag="Q2")
            fin = pers.tile([128, NQ], f16, tag="fin")

            nc.vector.memset(acc[:], 0.0)
            # meta on the ACT HW DGE ring so it does not delay J; J then A
            # sequentially on the SP ring (one big-row stream peaks at
            # ~430GB/s when alone; concurrent streams share it).
            nc.scalar.dma_start(metatile[:], meta[:])
            nc.sync.dma_start(J[:], jd[:])
            nc.sync.dma_start(A[:], ad[:])

            t3v = metatile[:, 0:MROW]
            BIAS0 = MROW
            biasv = metatile[:, BIAS0:BIAS0 + 7]   # pair0..2, sing0..2, mini
            INT0 = BIAS0 + 7
            # ints: pair joffs 0..2 | single joffs 3..5 | mini jB 6 |
            #       mini jA 7 | gjoff 8
            _, dvv = nc.values_load_multi_w_load_instructions(
                metatile[0:1, INT0:INT0 + 8].bitcast(i32),
                engines=(ET.DVE,), min_val=0, max_val=FREE + CENT + OFFB,
                skip_runtime_bounds_check=True)
            pv, sv, mv = dvv[0:3], dvv[3:6], dvv[6:8]
            gval = nc.values_load(
                metatile[0:1, INT0 + 8:INT0 + 9].bitcast(i32),
                engines=(ET.Activation,), min_val=FREE,
                max_val=FREE + CENT + OFFB,
                skip_runtime_bounds_check=True)

            J_A1 = bass.AP(J.tensor, CENT1, P1)
            J_A2 = bass.AP(J.tensor, CENT1, P2B)
            a_A1 = bass.AP(A.tensor, CENT1, P1)

            # ---- mini + G first: they only need J/meta, fill the DMA
            # shadow on both engines (v/ms parts emitted later) ----
            md = gp.tile([128, MROW], f16, tag="md")
            nc.vector.tensor_tensor(
                md[:], bass.AP(J.tensor, mv[0], PM),
                bass.AP(J.tensor, mv[1], PM), OP.subtract)
            mq = gp.tile([128, MROW], f16, tag="mq")
            nc.vector.tensor_tensor(mq[:], md[:], md[:], OP.mult)
            me = gp.tile([128, MROW], f16, tag="me")
            nc.scalar.activation(me[:], mq[:], AF.Exp, scale=-S2,
                                 bias=biasv[:, 6:7],
                                 accum_out=acc[:, 6:7])
            qg = gp.tile([128, MROW], f16, tag="qg")
            nc.scalar.activation(
                qg[:], bass.AP(J.tensor, gval, PM), AF.Square, scale=SQ)
            ag = gp.tile([128, MROW], f32, tag="ag")
            nc.vector.scalar_tensor_tensor(
                ag[:], qg[:], -1.0, t3v[:, 0:MROW], OP.mult, OP.add)
            eg = gp.tile([128, MROW], f16, tag="eg")
            nc.scalar.activation(eg[:], ag[:], AF.Exp,
                                 accum_out=acc[:, 10:11])

            def pair_slot(j, first):
                dt = dp2.tile([128, 2, BH, ROW], f16, tag="d2")
                nc.vector.tensor_tensor(
                    dt[:], bass.AP(J.tensor, pv[j], P2), J_A2, OP.subtract)
                qt = qp2.tile([128, 2, BH, ROW], f16, tag="q2")
                nc.scalar.activation(qt[:], dt[:], AF.Square, scale=SQ)
                et = ep2.tile([128, 2, BH, ROW], f16, tag="e2")
                nc.scalar.activation(et[:], qt[:], AF.Exp, scale=-1.0,
                                     bias=biasv[:, j:j + 1],
                                     accum_out=acc[:, j:j + 1])
                if first:
                    nc.vector.tensor_tensor(
                        Q2[:].rearrange("p (a b c) -> p a b c", a=2, b=BH),
                        et[:], bass.AP(A.tensor, pv[j], P2), OP.mult)
                else:
                    vt = vp2.tile([128, 2, BH, ROW], f16, tag="v2")
                    nc.vector.tensor_tensor(
                        vt[:], et[:], bass.AP(A.tensor, pv[j], P2), OP.mult)
                    nc.vector.tensor_tensor(
                        Q2[:], Q2[:],
                        vt[:].rearrange("p a b c -> p (a b c)"), OP.add)

            def single_slot(j, mode):
                # mode: "init" -> v writes Q1; "add" -> Q1 += v;
                #       "stt" -> bypass Q1, STT-accum into its own column
                dt = dp1.tile([128, BH, ROW], f16, tag="d1")
                nc.vector.tensor_tensor(
                    dt[:], bass.AP(J.tensor, sv[j], P1), J_A1, OP.subtract)
                qt = qp1.tile([128, BH, ROW], f16, tag="q1")
                if j == 0:
                    nc.vector.tensor_tensor(qt[:], dt[:], dt[:], OP.mult)
                    escale = -S2
                else:
                    nc.scalar.activation(qt[:], dt[:], AF.Square, scale=SQ)
                    escale = -1.0
                et = ep1.tile([128, BH, ROW], f16, tag="e1")
                nc.scalar.activation(et[:], qt[:], AF.Exp, scale=escale,
                                     bias=biasv[:, NPAIR + j:NPAIR + j + 1],
                                     accum_out=acc[:, NPAIR + j:NPAIR + j + 1])
                if mode == "init":
                    nc.vector.tensor_tensor(
                        Q1[:].rearrange("p (a b) -> p a b", a=BH),
                        et[:], bass.AP(A.tensor, sv[j], P1), OP.mult)
                    return
                vt = vp1.tile([128, BH, ROW], f16, tag="v1")
                nc.vector.tensor_tensor(
                    vt[:], et[:], bass.AP(A.tensor, sv[j], P1), OP.mult)
                if mode == "add":
                    nc.vector.tensor_tensor(
                        Q1[:], Q1[:],
                        vt[:].rearrange("p a b -> p (a b)"), OP.add)
                else:
                    st = gp.tile([128, BH, ROW], f16, tag="st")
                    nc.vector.scalar_tensor_tensor(
                        st[:], vt[:], 1.0, a_A1, OP.mult, OP.mult,
                        accum_out=acc[:, 11:12])

            single_slot(0, "init")
            pair_slot(0, True)
            single_slot(1, "add")
            pair_slot(1, False)
            pair_slot(2, False)

            # mini product part (needs A)
            mvt = gp.tile([128, MROW], f16, tag="mv")
            nc.vector.tensor_tensor(
                mvt[:], me[:], bass.AP(A.tensor, mv[0], PM), OP.mult)
            ms = gp.tile([128, MROW], f16, tag="ms")
            nc.vector.scalar_tensor_tensor(
                ms[:], mvt[:], 1.0, bass.AP(A.tensor, mv[1], PM),
                OP.mult, OP.mult, accum_out=acc[:, 9:10])

            # collapse pair lanes and fold into Q1 while single2 drains
            nc.vector.tensor_tensor(Q2[:, 0:NQ], Q2[:, 0:NQ],
                                    Q2[:, NQ:2 * NQ], OP.add)
            single_slot(2, "stt")
            nc.vector.tensor_tensor(Q1[:], Q1[:], Q2[:, 0:NQ], OP.add)

            # ---- final: col8 = sum aA * Q1 ----
            nc.vector.scalar_tensor_tensor(
                fin[:].rearrange("p (a b) -> p a b", a=BH, b=ROW),
                Q1[:].rearrange("p (a b) -> p a b", a=BH, b=ROW),
                1.0, a_A1, OP.mult, OP.mult, accum_out=acc[:, 8:9])

            nc.sync.dma_start(out[:], acc[:])
    nc.compile()
    return nc


def _host_tables(sample, spacing):
    """Per-core meta arrays."""
    sp = np.asarray(spacing, dtype=np.float64)[:, 0]
    pairs, singles = _pair_single_slots()

    # t3 = ln(noob) - 0.5*msq_center (NEG where noob == 0), central packing
    h = np.arange(H)[:, None, None]
    w = np.arange(W)[None, :, None]
    d = np.arange(D)[None, None, :]
    msq_c = ((sp[0] * h) ** 2 + (sp[1] * w) ** 2 + (sp[2] * d) ** 2) / SXY ** 2
    cnt = ((np.minimum(h, RH) + np.minimum(H - 1 - h, RH) + 1)
           * (np.minimum(w, RW) + np.minimum(W - 1 - w, RW) + 1)
           * (np.minimum(d, RD) + np.minimum(D - 1 - d, RD) + 1))
    noob = (2 * RH + 1) * (2 * RW + 1) * (2 * RD + 1) - cnt
    t3full = np.where(noob > 0, np.log(np.maximum(noob, 1)) - 0.5 * msq_c, NEG)
    t3b = np.empty((128, BH, BW, D), np.float32)
    for hb in range(NHB):
        for wb in range(NWB):
            t3b[hb * NWB + wb] = t3full[hb * BH:(hb + 1) * BH,
                                        wb * BW:(wb + 1) * BW, :]
    t3flat = t3b.reshape(128, BH * BW * D)

    def bias_of(dh, dw, dd):
        msq = ((sp[0] * dh) ** 2 + (sp[1] * dw) ** 2
               + (sp[2] * dd) ** 2) / SXY ** 2
        return -0.5 * msq

    metas = []
    for c in range(NCORES):
        m = np.zeros((128, NMETA), np.float32)
        # G/mini slice: h-row c//2, col half c%2 of the 272-wide flat row
        sl = (c // 2) * SW * SD + (c % 2) * MROW
        # map on-chip slice positions (d incl halo) to central t3 values
        t3s = np.full((128, MROW), NEG, np.float32)
        r0 = c // 2
        base = (c % 2) * MROW
        for i in range(MROW):
            wcol, dcol = divmod(base + i, SD)
            if 1 <= dcol <= D:
                t3s[:, i] = t3flat[:, (r0 * BW + wcol) * D + (dcol - 1)]
        m[:, 0:MROW] = t3s

        B0 = MROW
        joff_p = np.zeros(3, np.int32)
        joff_s = np.zeros(3, np.int32)
        for j in range(NPAIR):
            dh, dw = pairs[3 * c + j]
            j0 = dh * SW * SD + (RW + dw) * SD + (RD - 1)   # dd=-1 lane
            assert j0 % 2 == 0
            joff_p[j] = j0                                  # copy-0 coords
            m[:, B0 + j] = bias_of(dh, dw, 1)
        for j in range(NSING):
            dh, dw = singles[3 * c + j]
            j0 = dh * SW * SD + (RW + dw) * SD + RD         # dd=0
            assert j0 % 2 == 1
            joff_s[j] = FREE + j0 - 1                       # copy-1 coords
            m[:, B0 + NPAIR + j] = bias_of(dh, dw, 0)
        m[:, B0 + 6] = bias_of(0, 0, 1)                     # mini
        I0 = B0 + 7
        m[0, I0:I0 + 3] = joff_p.view(np.float32)
        m[0, I0 + 3:I0 + 6] = joff_s.view(np.float32)
        # mini: B = central+1 (dd=+1) in copy-0, A = central in copy-1
        mb = CENT + 1 + sl
        ma = CENT1 + sl
        assert mb % 2 == 0 and ma % 2 == 0
        m[0, I0 + 6] = np.int32(mb).view(np.float32)
        m[0, I0 + 7] = np.int32(ma).view(np.float32)
        m[0, I0 + 8] = np.int32(CENT1 + sl).view(np.float32)  # gjoff
        metas.append(m)
    return metas


def _host_inputs(y_hat_softmax, sample, spacing):
    y0 = np.asarray(y_hat_softmax, dtype=np.float32)[0, 0]
    I = np.asarray(sample, dtype=np.float32)[0, 0]
    jd = _dual_f16(_pack(I, BIG), 0.0)
    ad = _dual_f16(_pack(1.0 - 2.0 * y0, 1.0), 1.0)
    metas = _host_tables(sample, spacing)
    return [{"jd": jd, "ad": ad, "meta": metas[c]} for c in range(NCORES)]


def kernel(y_hat_softmax, sample, spacing):
    from concourse.bass_utils import run_bass_kernel_spmd

    in_maps = _host_inputs(y_hat_softmax, sample, spacing)
    nc = _build_nc()
    res = run_bass_kernel_spmd(nc, in_maps, core_ids=list(range(NCORES)))
    total = 0.0
    for r in res.results:
        o = r["out"].astype(np.float64)
        # cols 0..6 = sum E per slot (pairs, singles, mini); col8 = final
        # sum aA*Q1; col9 = mini product sum; col10 = G; col11 = single2
        # product sum
        total += (o[:, 0:7].sum() - o[:, 8].sum() - o[:, 9].sum()
                  + o[:, 10].sum() - o[:, 11].sum())
    return np.array(total / DENOM, dtype=np.float32)


if __name__ == "__main__":
    rng = np.random.default_rng(0)
    logits = rng.standard_normal((1, 2, H, W, D)).astype(np.float32)
    e = np.exp(logits - logits.max(axis=1, keepdims=True))
    yh = (e / e.sum(axis=1, keepdims=True)).astype(np.float32)
    smp = rng.standard_normal((1, 1, H, W, D)).astype(np.float32)
    spc = rng.uniform(0.5, 2.0, (3, 1)).astype(np.float32)
    print(kernel(yh, smp, spc))
